# revision 14
# baseline (speedup 1.0000x reference)
"""Compositional attention kernel for Trainium2, 8-core SPMD.

Sharding: core c handles batch b = c // 4 and search-heads {2*(c%4), 2*(c%4)+1}
end-to-end (tensor-parallel over the S=8 search heads x data-parallel over
B=2).  Each core computes a partial y = out_heads @ w_out[head_rows] for its
batch in fp16; the host sums the 4 partials per batch in fp32.

All matmuls run in fp16 with fp32 PSUM accumulation.  Schedule notes:
- ~32 warm-up matmuls (ident@ident) run during the input-DMA wait so the PE
  HAM clock-gate opens (1.2->2.4 GHz) before the first real matmul.
- Input DMAs are spread over 4 queues (sync/scalar/vector/gpsimd).
- Z is computed with an in-place pair-tree (8 DVE adds + 8 ones-matmuls per
  i-quarter) for BOTH heads; the Z row [1,512] is turned into the per-i
  partition layout with 4 tiny PE transposes (no DRAM round-trip).
- DVE work is batched 4-wide (rv16/un/rqw/uf2 copies) and stage-2 runs at
  half-head width to amortize per-instruction overhead.
- Head-0's trailing work (AV c2/c3, Z tree, un transposes, stage-2) drains
  11 ops/jt inside head-1's score/exp stream; head-1's trail is pipelined in
  the tail with the out-projection so the PE never idles long enough to
  re-throttle.
"""

import sys

import numpy as np

for _p in ("/opt/trn_rl_repo", "/root/.axon_site/_ro/trn_rl_repo"):
    if _p not in sys.path:
        sys.path.append(_p)

import concourse.bass as bass  # noqa: F401
import concourse.mybir as mybir
import concourse.tile as tile
from concourse import bacc
from concourse.bass_utils import run_bass_kernel_spmd
from concourse.masks import make_identity

S, R, DH = 8, 2, 64
B, N, DIM = 2, 2048, 512
P = 128
NT = N // P  # 16 j-tiles
KC = DIM // P  # 4 contraction chunks of x
NCORES = 8
HPC = 2  # heads per core

F32 = mybir.dt.float32
F16 = mybir.dt.float16
AF = mybir.ActivationFunctionType
ALU = mybir.AluOpType

N_WARM = 24


def _emit(tc, xt, wq, wk, wv, wqr, wkt, wo, y):
    from contextlib import ExitStack

    nc = tc.nc
    with ExitStack() as ctx:
        cpool = ctx.enter_context(tc.tile_pool(name="const", bufs=1))
        xp = ctx.enter_context(tc.tile_pool(name="xp", bufs=1))
        qp = ctx.enter_context(tc.tile_pool(name="qp", bufs=1))
        sb = ctx.enter_context(tc.tile_pool(name="sb", bufs=1))
        hd = ctx.enter_context(tc.tile_pool(name="hd", bufs=2))
        # PSUM: st 2x[128,1024]f32 = 4 banks, av 1x[128,1024]f32 = 2 banks,
        # wk 2x[<=2KB] = 2 banks  -> 8 banks exactly.
        pst = ctx.enter_context(tc.tile_pool(name="pst", bufs=2, space="PSUM"))
        pav = ctx.enter_context(tc.tile_pool(name="pav", bufs=1, space="PSUM"))
        pwk = ctx.enter_context(tc.tile_pool(name="pwk", bufs=2, space="PSUM"))

        # ---- constants ----
        ones16 = cpool.tile([P, 1], F16, name="ones16")
        nc.gpsimd.memset(ones16, 1.0)
        ident = cpool.tile([P, P], F16, name="ident")
        make_identity(nc, ident)

        # ---- PE warm-up: keep the PE busy during the DMA wait so the HAM
        # clock-gate opens before the first projection matmul.  Tiny
        # ones-matmuls start as soon as the memset lands; ident matmuls
        # (128-col streams) follow once make_identity finishes. ----
        pw1 = pav.tile([1, 1], F32, tag="av", name="pw1")
        for _ in range(24):
            nc.tensor.matmul(
                pw1, ones16, ones16, start=True, stop=True,
                skip_group_check=True,
            )
        pwarm = pav.tile([P, P], F32, tag="av", name="pwarm")
        for _ in range(N_WARM):
            nc.tensor.matmul(
                pwarm, ident, ident, start=True, stop=True,
                skip_group_check=True,
            )

        def tick(n):
            """Standalone ident weight-loads: PE-array activity with no PSUM
            hazards.  The PE's LDWEIGHTS pull-ahead lets these run during
            stalls of in-flight matmuls, keeping the HAM clock-gate open
            through DMA waits and ACT-paced stretches."""
            for _ in range(n):
                nc.tensor.ldweights(weights=ident)

        # x^T chunks spread over the 3 DMA-capable queues; weights behind.
        xT = xp.tile([P, KC, N], F16, tag="x", name="xT")
        nc.sync.dma_start(xT[:, 0, :], xt[0:P, :])
        wq_sb = cpool.tile([P, KC, P], F16, name="wq_sb")
        nc.scalar.dma_start(wq_sb, wq.rearrange("(kc p) m -> p kc m", p=P))
        nc.gpsimd.dma_start(xT[:, 2, :], xt[2 * P : 3 * P, :])
        nc.sync.dma_start(xT[:, 3, :], xt[3 * P : 4 * P, :])
        nc.scalar.dma_start(xT[:, 1, :], xt[P : 2 * P, :])
        wk_sb = cpool.tile([P, KC, P], F16, name="wk_sb")
        nc.scalar.dma_start(wk_sb, wk.rearrange("(kc p) m -> p kc m", p=P))
        wv_sb = cpool.tile([P, KC, P], F16, name="wv_sb")
        nc.gpsimd.dma_start(wv_sb, wv.rearrange("(kc p) m -> p kc m", p=P))
        wqr_sb = cpool.tile([P, KC, P], F16, name="wqr_sb")
        nc.sync.dma_start(wqr_sb, wqr.rearrange("(kc p) m -> p kc m", p=P))
        wkt_sb = cpool.tile([2 * DH, DH], F16, name="wkt_sb")
        nc.scalar.dma_start(wkt_sb[0:DH, :], wkt[:, :])
        nc.scalar.dma_start(wkt_sb[DH : 2 * DH, :], wkt[:, :])
        wo_sb = cpool.tile([P, DIM], F16, name="wo_sb")
        nc.gpsimd.dma_start(wo_sb, wo[:, :])

        # ---- persistent SBUF tiles ----
        sqT = qp.tile([P, N], F16, tag="q", name="sqT")
        skT = sb.tile([P, N], F16, name="skT")
        rqT = sb.tile([P, N], F16, name="rqT")
        rv16 = sb.tile([P, NT, P], F16, name="rv16")
        ET0 = sb.tile([P, NT, N], F16, name="ET0")
        ET1 = sb.tile([P, NT - 4, N], F16, name="ET1")
        uT16 = [sb.tile([P, N], F16, name=f"uT16_{h}") for h in range(HPC)]
        un = [sb.tile([P, NT, R, DH], F16, name=f"un{h}") for h in range(HPC)]
        rqw = [sb.tile([P, NT, DH], F16, name=f"rqw{h}") for h in range(HPC)]
        uc2 = sb.tile([P, NT, P], F16, name="uc2")
        gdt = [sb.tile([P, NT], F32, name=f"gdt{h}") for h in range(HPC)]
        zit = [sb.tile([P, NT], F32, name=f"zit{h}") for h in range(HPC)]
        state = {"ET1d": None, "uf2": None}

        def et(h, jt):
            if h == 0:
                return ET0, jt
            if jt < NT - 4:
                return ET1, jt
            return state["ET1d"], jt - (NT - 4)

        # ---- q/k projections: kc-outer so matmuls start on the first
        # arriving x chunk ----
        def proj_icp(wsb, dst, icp, ticks=0):
            pp = pst.tile([P, 1024], F32, tag="st", name="pp")
            for kc in range(KC):
                for half in range(2):
                    i0 = icp * 1024 + half * 512
                    nc.tensor.matmul(
                        pp[:, half * 512 : (half + 1) * 512],
                        wsb[:, kc, :],
                        xT[:, kc, i0 : i0 + 512],
                        start=(kc == 0),
                        stop=(kc == KC - 1),
                        skip_group_check=True,
                    )
                tick(ticks)
            nc.scalar.copy(out=dst[:, icp * 1024 : (icp + 1) * 1024], in_=pp)

        proj_icp(wq_sb, sqT, 0, ticks=6)
        proj_icp(wk_sb, skT, 0, ticks=6)
        proj_icp(wq_sb, sqT, 1)
        proj_icp(wk_sb, skT, 1)

        def rv_chunk(ic):
            pv = pwk.tile([P, 512], F32, tag="wk", name="pv")
            for kc in range(KC):
                nc.tensor.matmul(
                    pv,
                    wv_sb[:, kc, :],
                    xT[:, kc, ic * 512 : (ic + 1) * 512],
                    start=(kc == 0),
                    stop=(kc == KC - 1),
                )
            rvT_c = hd.tile([P, 512], F16, tag="rvT", name="rvT_c")
            nc.vector.tensor_copy(out=rvT_c, in_=pv)
            pt4 = pwk.tile([P, 512], F16, tag="wk", name="pt4")
            for t in range(4):
                nc.tensor.transpose(
                    pt4[:, t * P : (t + 1) * P], rvT_c[:, t * P : (t + 1) * P],
                    ident,
                )
            nc.vector.tensor_copy(out=rv16[:, ic * 4 : (ic + 1) * 4, :], in_=pt4)

        def rq_chunk(icp):
            for half in range(2):
                i0 = icp * 1024 + half * 512
                pp = pwk.tile([P, 512], F32, tag="wk", name="prq")
                for kc in range(KC):
                    nc.tensor.matmul(
                        pp,
                        wqr_sb[:, kc, :],
                        xT[:, kc, i0 : i0 + 512],
                        start=(kc == 0),
                        stop=(kc == KC - 1),
                    )
                nc.vector.tensor_copy(out=rqT[:, i0 : i0 + 512], in_=pp)

        rv_chunk(0)  # av(h0, jt=0) needs rv16[0..3] early

        def rqw_four(h, b4):
            """4 rqw matmuls into one PSUM tile, one batched copy."""
            hs = slice(h * DH, (h + 1) * DH)
            pq = pwk.tile([P, 4 * DH], F32, tag="wk", name="pq")
            for t in range(4):
                it = b4 * 4 + t
                nc.tensor.matmul(
                    pq[:, t * DH : (t + 1) * DH],
                    rqT[hs, it * P : (it + 1) * P],
                    wkt_sb[hs, :],
                    start=True,
                    stop=True,
                    skip_group_check=True,
                )
            nc.vector.tensor_copy(out=rqw[h][:, b4 * 4 : (b4 + 1) * 4, :], in_=pq)

        def stage2_pre(h, hf):
            """sims + gating logits for half hf (8 it's); batched DVE."""
            its = slice(8 * hf, 8 * hf + 8)
            sims = []
            for r in range(R):
                prod = hd.tile([P, 8, DH], F16, tag="prod", name="prod")
                nc.vector.tensor_tensor(
                    prod, un[h][:, its, r, :], rqw[h][:, its, :], ALU.mult
                )
                s_ = hd.tile([P, 8], F32, tag=f"sims{r}", name=f"sims{r}")
                nc.vector.tensor_reduce(
                    s_, prod, axis=mybir.AxisListType.X, op=ALU.add
                )
                sims.append(s_)
            gds = gdt[h][:, its]
            nc.vector.tensor_tensor(gds, sims[0], sims[1], ALU.subtract)
            nc.vector.tensor_tensor(gds, gds, zit[h][:, its], ALU.mult)

        def stage2_post(h, hf):
            """Gating exp + uc writes for half hf (8 it's)."""
            its = slice(8 * hf, 8 * hf + 8)
            p0 = hd.tile([P, 8], F32, tag="p0", name="p0")
            nc.scalar.activation(p0, gdt[h][:, its], AF.Exp)
            w_ = hd.tile([P, 8], F32, tag="w_", name="w_")
            nc.vector.tensor_scalar_add(w_, p0, 1.0)
            nc.vector.reciprocal(w_, w_)
            nc.vector.tensor_tensor(w_, w_, zit[h][:, its], ALU.mult)
            a0z = hd.tile([P, 8], F32, tag="a0z", name="a0z")
            nc.vector.tensor_tensor(a0z, w_, p0, ALU.mult)
            ucs = uc2[:, its, h * DH : (h + 1) * DH]
            nc.vector.tensor_tensor(
                ucs, un[h][:, its, 0, :],
                a0z[:, :, None].to_broadcast((P, 8, DH)), ALU.mult,
            )
            t1 = hd.tile([P, 8, DH], F16, tag="t1", name="t1")
            nc.vector.tensor_tensor(
                t1, un[h][:, its, 1, :],
                w_[:, :, None].to_broadcast((P, 8, DH)), ALU.mult,
            )
            nc.vector.tensor_tensor(ucs, ucs, t1, ALU.add)

        def avt_pass(h, c):
            pu = pwk.tile([P, 512], F32, tag="wk", name=f"pu{h}{c}")
            for jt in range(NT):
                def mm(jt=jt, c=c, pu=pu):
                    t_, lj = et(h, jt)
                    nc.tensor.matmul(
                        pu,
                        rv16[:, jt, :],
                        t_[:, lj, c * 512 : (c + 1) * 512],
                        start=(jt == 0),
                        stop=(jt == NT - 1),
                        skip_group_check=True,
                    )
                yield mm
            def cp(c=c, pu=pu):
                nc.vector.tensor_copy(
                    out=uT16[h][:, c * 512 : (c + 1) * 512], in_=pu
                )
            yield cp

        def unT_four(h, b4, on_scalar=False):
            """4 u transposes into one PSUM tile + one batched copy."""
            pt2 = pwk.tile([P, 512], F16, tag="wk", name="pt2")
            for t in range(4):
                it = b4 * 4 + t
                def tr(t=t, it=it, pt2=pt2):
                    nc.tensor.transpose(
                        pt2[:, t * P : (t + 1) * P],
                        uT16[h][:, it * P : (it + 1) * P], ident,
                    )
                yield tr
            def cp(pt2=pt2, b4=b4):
                dst = un[h][:, b4 * 4 : (b4 + 1) * 4]
                if on_scalar:
                    nc.scalar.copy(out=dst, in_=pt2)
                else:
                    nc.vector.tensor_copy(out=dst, in_=pt2)
            yield cp

        def z_adds(h, q, eng):
            """Level-1 pair adds (in place) for i-quarter q on engine eng."""
            cs = slice(q * 512, (q + 1) * 512)
            for t in range(8):
                def add(t=t, cs=cs, eng=eng):
                    ta, a = et(h, 2 * t)
                    tb, b_ = et(h, 2 * t + 1)
                    eng.tensor_tensor(
                        ta[:, a, cs], ta[:, a, cs], tb[:, b_, cs], ALU.add
                    )
                yield add

        def z_extract(h, q, pz):
            """Z row [1,512] -> zit columns via 4 tiny PE transposes."""
            zrow16 = hd.tile([1, 512], F16, tag="zrow", name="zrow16")
            def zcp(pz=pz, zrow16=zrow16):
                nc.vector.tensor_copy(out=zrow16, in_=pz)
            yield zcp
            pzt = pwk.tile([P, 8], F16, tag="wk", name="pzt")
            for t in range(4):
                def ztr(t=t, pzt=pzt, zrow16=zrow16):
                    nc.tensor.transpose(
                        pzt[:, 2 * t : 2 * t + 1],
                        zrow16[:, t * P : (t + 1) * P],
                        ident[0:1, 0:1],
                    )
                yield ztr
            def zfin(pzt=pzt, h=h, q=q):
                nc.vector.tensor_copy(
                    out=zit[h][:, q * 4 : (q + 1) * 4], in_=pzt[:, 0:8:2]
                )
            yield zfin

        def z_mms(h, q):
            """8 ones-matmuls over pair-summed tiles + extraction."""
            cs = slice(q * 512, (q + 1) * 512)
            pz = pwk.tile([1, 512], F32, tag="wk", name=f"pz{h}{q}")
            for g in range(8):
                def mm(g=g, pz=pz, cs=cs):
                    t_, lj = et(h, 2 * g)
                    nc.tensor.matmul(
                        pz,
                        ones16,
                        t_[:, lj, cs],
                        start=(g == 0),
                        stop=(g == 7),
                    )
                yield mm
            yield from z_extract(h, q, pz)

        def z_pass16(h, q):
            """16 ones-matmuls over the raw exp tiles (no DVE adds) +
            extraction; used in the tail where the PE has slack."""
            cs = slice(q * 512, (q + 1) * 512)
            pz = pwk.tile([1, 512], F32, tag="wk", name=f"pzp{h}{q}")
            for jt in range(NT):
                def mm(jt=jt, pz=pz, cs=cs):
                    t_, lj = et(h, jt)
                    nc.tensor.matmul(
                        pz,
                        ones16,
                        t_[:, lj, cs],
                        start=(jt == 0),
                        stop=(jt == NT - 1),
                    )
                yield mm
            yield from z_extract(h, q, pz)

        def z_recip(h, hf):
            nc.vector.reciprocal(
                zit[h][:, 8 * hf : 8 * hf + 8], zit[h][:, 8 * hf : 8 * hf + 8]
            )

        def paired(g1, g2):
            """Interleave two op generators (they use the two wk buffers)."""
            while True:
                a = next(g1, None)
                b = next(g2, None)
                if a is None and b is None:
                    return
                if a is not None:
                    yield a
                if b is not None:
                    yield b

        def trail0():
            """Head-0 trailing work, drained inside head-1's stream."""
            # un transposes for it 0-7 (need only the av01 half of uT16)
            yield from unT_four(0, 0)
            yield from unT_four(0, 1)
            # q0/q1 pair adds: av01 already consumed those columns
            yield from z_adds(0, 0, nc.vector)
            yield from z_adds(0, 1, nc.vector)
            # AV c2/c3 read the raw exp tiles (cols 1024:2048)
            yield from paired(avt_pass(0, 2), avt_pass(0, 3))
            yield from z_mms(0, 0)
            yield from z_mms(0, 1)
            yield lambda: z_recip(0, 0)
            yield from z_adds(0, 2, nc.vector)
            yield from z_adds(0, 3, nc.vector)
            yield from z_mms(0, 2)
            yield from z_mms(0, 3)
            yield lambda: z_recip(0, 1)
            yield from unT_four(0, 2)
            yield from unT_four(0, 3)
            yield lambda: stage2_pre(0, 0)
            yield lambda: stage2_pre(0, 1)
            yield lambda: stage2_post(0, 0)
            yield lambda: stage2_post(0, 1)

        def out_a_four(b4):
            """4 uc2 transposes -> uf2 with one batched copy."""
            pf = pwk.tile([P, 512], F16, tag="wk", name="pf")
            for t in range(4):
                it = b4 * 4 + t
                nc.tensor.transpose(
                    pf[:, t * P : (t + 1) * P], uc2[:, it], ident
                )
            nc.scalar.copy(
                out=state["uf2"][:, b4 * 512 : (b4 + 1) * 512], in_=pf
            )

        def out_b(it):
            """Out-projection matmul + ysb copy (alternating engines) + DMA."""
            py = pst.tile([P, DIM], F32, tag="st", name="py")
            nc.tensor.matmul(
                py,
                state["uf2"][:, it * P : (it + 1) * P],
                wo_sb,
                start=True,
                stop=True,
            )
            ysb = hd.tile([P, DIM], F16, tag="ysb", name="ysb")
            nc.scalar.copy(out=ysb, in_=py)
            q = nc.sync if it % 2 == 0 else nc.gpsimd
            q.dma_start(y[it * P : (it + 1) * P, :], ysb)

        # -------- main per-head jt-loops --------
        trail = None
        for h in range(HPC):
            hs = slice(h * DH, (h + 1) * DH)
            if h == 1:
                trail = trail0()
                state["ET1d"] = xp.tile([P, 4, N], F16, tag="x", name="ET1d")
                state["uf2"] = qp.tile([P, N], F16, tag="q", name="uf2")
            av01 = pav.tile([P, 1024], F32, tag="av", name=f"av01_{h}")
            for jt in range(NT):
                th, lj = et(h, jt)
                for icp in range(2):
                    st = pst.tile([P, 1024], F32, tag="st", name="st")
                    for half in range(2):
                        i0 = icp * 1024 + half * 512
                        nc.tensor.matmul(
                            st[:, half * 512 : (half + 1) * 512],
                            skT[hs, jt * P : (jt + 1) * P],
                            sqT[hs, i0 : i0 + 512],
                            start=True,
                            stop=True,
                            skip_group_check=True,
                        )
                    nc.scalar.activation(
                        th[:, lj, icp * 1024 : (icp + 1) * 1024], st, AF.Exp
                    )
                if jt >= 2:
                    ajt = jt - 2
                    ta, la = et(h, ajt)
                    for c in range(2):
                        nc.tensor.matmul(
                            av01[:, c * 512 : (c + 1) * 512],
                            rv16[:, ajt, :],
                            ta[:, la, c * 512 : (c + 1) * 512],
                            start=(ajt == 0),
                            stop=False,
                            skip_group_check=True,
                        )
                if h == 0:
                    if jt in (1, 3, 5):
                        rv_chunk((jt + 1) // 2)
                    elif jt in (7, 9):
                        rq_chunk((jt - 7) // 2)
                    elif 10 <= jt <= 13:
                        b0 = (jt - 10) * 2
                        rqw_four(b0 // 4, b0 % 4)
                        rqw_four((b0 + 1) // 4, (b0 + 1) % 4)
                        tick(4)
                    else:
                        # sparse jt: keep the PE-activity window busy so the
                        # HAM clock-gate doesn't flap mid-stream
                        tick(8)
                else:
                    for _ in range(9):
                        op = next(trail, None)
                        if op is None:
                            break
                        op()

            def av01_tail(h=h, av01=av01):
                for ajt in (NT - 2, NT - 1):
                    ta, la = et(h, ajt)
                    for c in range(2):
                        nc.tensor.matmul(
                            av01[:, c * 512 : (c + 1) * 512],
                            rv16[:, ajt, :],
                            ta[:, la, c * 512 : (c + 1) * 512],
                            start=False,
                            stop=(ajt == NT - 1),
                            skip_group_check=True,
                        )
                if h == 1:
                    nc.scalar.copy(out=uT16[h][:, 0:1024], in_=av01)
                else:
                    nc.vector.tensor_copy(out=uT16[h][:, 0:1024], in_=av01)

            if h == 0:
                av01_tail()
            else:
                for op in trail:
                    op()
                # avt c2/c3 only read exp cols 1024:2048 and jt<=13 early, so
                # the PE stays busy while the last two exps finish; av01's
                # tail (which reads cols 0:1024 of jt14/15) lands behind it.
                for op in paired(avt_pass(1, 2), avt_pass(1, 3)):
                    op()
                av01_tail()

        # -------- tail: head-1 trailing work pipelined with the
        # out-projection; PE-dense ordering to avoid HAM re-throttle --------
        for op in paired(z_pass16(1, 0), z_pass16(1, 1)):
            op()
        for op in unT_four(1, 0, on_scalar=True):
            op()
        for op in unT_four(1, 1, on_scalar=True):
            op()
        z_recip(1, 0)
        stage2_pre(1, 0)
        for op in paired(z_pass16(1, 2), z_pass16(1, 3)):
            op()
        for op in unT_four(1, 2, on_scalar=True):
            op()
        for op in unT_four(1, 3, on_scalar=True):
            op()
        z_recip(1, 1)
        stage2_post(1, 0)
        tick(6)
        out_a_four(0)
        out_a_four(1)
        stage2_pre(1, 1)
        tick(4)
        for it in range(0, 8):
            out_b(it)
            tick(2)
        stage2_post(1, 1)
        tick(6)
        out_a_four(2)
        out_a_four(3)
        for it in range(8, 16):
            out_b(it)
            tick(2)


def build_program():
    nc = bacc.Bacc(None, target_bir_lowering=False)
    xt = nc.declare_dram_parameter("xt", [DIM, N], F16, isOutput=False)
    wq = nc.declare_dram_parameter("wq", [DIM, P], F16, isOutput=False)
    wk = nc.declare_dram_parameter("wk", [DIM, P], F16, isOutput=False)
    wv = nc.declare_dram_parameter("wv", [DIM, P], F16, isOutput=False)
    wqr = nc.declare_dram_parameter("wqr", [DIM, P], F16, isOutput=False)
    wkt = nc.declare_dram_parameter("wkt", [DH, DH], F16, isOutput=False)
    wo = nc.declare_dram_parameter("wo", [P, DIM], F16, isOutput=False)
    y = nc.declare_dram_parameter("y", [N, DIM], F16, isOutput=True)

    with tile.TileContext(nc) as tc:
        _emit(tc, xt, wq, wk, wv, wqr, wkt, wo, y)
    nc.compile()
    return nc


_NC_CACHE = None


def _get_program():
    global _NC_CACHE
    if _NC_CACHE is None:
        _NC_CACHE = build_program()
    return _NC_CACHE


def make_in_maps(inputs):
    x = np.asarray(inputs["x"], dtype=np.float32)
    wq_s = np.asarray(inputs["wq_s"], dtype=np.float32)
    wk_s = np.asarray(inputs["wk_s"], dtype=np.float32)
    wv_r = np.asarray(inputs["wv_r"], dtype=np.float32)
    wq_r = np.asarray(inputs["wq_r"], dtype=np.float32)
    wk_ret = np.asarray(inputs["wk_ret"], dtype=np.float32)
    w_out = np.asarray(inputs["w_out"], dtype=np.float32)
    scale = np.float32(DH**-0.5)

    f16 = np.float16
    in_maps = []
    for c in range(NCORES):
        b, hp = divmod(c, NCORES // B)
        cols = slice(hp * P, (hp + 1) * P)
        in_maps.append(
            {
                "xt": np.ascontiguousarray(x[b].T).astype(f16),
                "wq": (np.ascontiguousarray(wq_s[:, cols]) * scale).astype(f16),
                "wk": np.ascontiguousarray(wk_s[:, cols]).astype(f16),
                "wv": wv_r.astype(f16),
                "wqr": (np.ascontiguousarray(wq_r[:, cols]) * scale).astype(f16),
                "wkt": np.ascontiguousarray(wk_ret.T).astype(f16),
                "wo": np.ascontiguousarray(w_out[hp * P : (hp + 1) * P, :]).astype(f16),
            }
        )
    return in_maps


def run(inputs, trace=False, **kw):
    res = run_bass_kernel_spmd(
        _get_program(), make_in_maps(inputs), list(range(NCORES)), trace=trace, **kw
    )
    out = np.zeros((B, N, DIM), np.float32)
    for c in range(NCORES):
        out[c // (NCORES // B)] += np.asarray(res.results[c]["y"], np.float32)
    return out, res


def kernel(**inputs):
    out, _ = run(inputs)
    return out


# revision 15
# speedup vs baseline: 1.0094x; 1.0094x over previous
"""Compositional attention kernel for Trainium2, 8-core SPMD.

Sharding: core c handles batch b = c // 4 and search-heads {2*(c%4), 2*(c%4)+1}
end-to-end (tensor-parallel over the S=8 search heads x data-parallel over
B=2).  Each core computes a partial y = out_heads @ w_out[head_rows] for its
batch in fp16; the host sums the 4 partials per batch in fp32.

All matmuls run in fp16 with fp32 PSUM accumulation.  Schedule notes:
- ~32 warm-up matmuls (ident@ident) run during the input-DMA wait so the PE
  HAM clock-gate opens (1.2->2.4 GHz) before the first real matmul.
- Input DMAs are spread over 4 queues (sync/scalar/vector/gpsimd).
- Z is computed with an in-place pair-tree (8 DVE adds + 8 ones-matmuls per
  i-quarter) for BOTH heads; the Z row [1,512] is turned into the per-i
  partition layout with 4 tiny PE transposes (no DRAM round-trip).
- DVE work is batched 4-wide (rv16/un/rqw/uf2 copies) and stage-2 runs at
  half-head width to amortize per-instruction overhead.
- Head-0's trailing work (AV c2/c3, Z tree, un transposes, stage-2) drains
  11 ops/jt inside head-1's score/exp stream; head-1's trail is pipelined in
  the tail with the out-projection so the PE never idles long enough to
  re-throttle.
"""

import sys

import numpy as np

for _p in ("/opt/trn_rl_repo", "/root/.axon_site/_ro/trn_rl_repo"):
    if _p not in sys.path:
        sys.path.append(_p)

import concourse.bass as bass  # noqa: F401
import concourse.mybir as mybir
import concourse.tile as tile
from concourse import bacc
from concourse.bass_utils import run_bass_kernel_spmd
from concourse.masks import make_identity

S, R, DH = 8, 2, 64
B, N, DIM = 2, 2048, 512
P = 128
NT = N // P  # 16 j-tiles
KC = DIM // P  # 4 contraction chunks of x
NCORES = 8
HPC = 2  # heads per core

F32 = mybir.dt.float32
F16 = mybir.dt.float16
AF = mybir.ActivationFunctionType
ALU = mybir.AluOpType

N_WARM = 24


def _emit(tc, xt, wq, wk, wv, wqr, wkt, wo, y):
    from contextlib import ExitStack

    nc = tc.nc
    with ExitStack() as ctx:
        cpool = ctx.enter_context(tc.tile_pool(name="const", bufs=1))
        xp = ctx.enter_context(tc.tile_pool(name="xp", bufs=1))
        qp = ctx.enter_context(tc.tile_pool(name="qp", bufs=1))
        sb = ctx.enter_context(tc.tile_pool(name="sb", bufs=1))
        hd = ctx.enter_context(tc.tile_pool(name="hd", bufs=2))
        # PSUM: st 2x[128,1024]f32 = 4 banks, av 1x[128,1024]f32 = 2 banks,
        # wk 2x[<=2KB] = 2 banks  -> 8 banks exactly.
        pst = ctx.enter_context(tc.tile_pool(name="pst", bufs=2, space="PSUM"))
        pav = ctx.enter_context(tc.tile_pool(name="pav", bufs=1, space="PSUM"))
        pwk = ctx.enter_context(tc.tile_pool(name="pwk", bufs=2, space="PSUM"))

        # ---- constants ----
        ones16 = cpool.tile([P, 1], F16, name="ones16")
        nc.gpsimd.memset(ones16, 1.0)
        ident = cpool.tile([P, P], F16, name="ident")
        make_identity(nc, ident)

        # ---- PE warm-up: keep the PE busy during the DMA wait so the HAM
        # clock-gate opens before the first projection matmul.  Tiny
        # ones-matmuls start as soon as the memset lands; ident matmuls
        # (128-col streams) follow once make_identity finishes. ----
        pw1 = pav.tile([1, 1], F32, tag="av", name="pw1")
        for _ in range(24):
            nc.tensor.matmul(
                pw1, ones16, ones16, start=True, stop=True,
                skip_group_check=True,
            )
        pwarm = pav.tile([P, P], F32, tag="av", name="pwarm")
        for _ in range(N_WARM):
            nc.tensor.matmul(
                pwarm, ident, ident, start=True, stop=True,
                skip_group_check=True,
            )

        def tick(n):
            """Standalone ident weight-loads: PE-array activity with no PSUM
            hazards.  The PE's LDWEIGHTS pull-ahead lets these run during
            stalls of in-flight matmuls, keeping the HAM clock-gate open
            through DMA waits and ACT-paced stretches."""
            for _ in range(n):
                nc.tensor.ldweights(weights=ident)

        # x^T chunks spread over the 3 DMA-capable queues; weights behind.
        xT = xp.tile([P, KC, N], F16, tag="x", name="xT")
        nc.sync.dma_start(xT[:, 0, :], xt[0:P, :])
        wq_sb = cpool.tile([P, KC, P], F16, name="wq_sb")
        nc.scalar.dma_start(wq_sb, wq.rearrange("(kc p) m -> p kc m", p=P))
        nc.gpsimd.dma_start(xT[:, 2, :], xt[2 * P : 3 * P, :])
        nc.sync.dma_start(xT[:, 3, :], xt[3 * P : 4 * P, :])
        nc.scalar.dma_start(xT[:, 1, :], xt[P : 2 * P, :])
        wk_sb = cpool.tile([P, KC, P], F16, name="wk_sb")
        nc.scalar.dma_start(wk_sb, wk.rearrange("(kc p) m -> p kc m", p=P))
        wv_sb = cpool.tile([P, KC, P], F16, name="wv_sb")
        nc.gpsimd.dma_start(wv_sb, wv.rearrange("(kc p) m -> p kc m", p=P))
        wqr_sb = cpool.tile([P, KC, P], F16, name="wqr_sb")
        nc.sync.dma_start(wqr_sb, wqr.rearrange("(kc p) m -> p kc m", p=P))
        wkt_sb = cpool.tile([2 * DH, DH], F16, name="wkt_sb")
        nc.scalar.dma_start(wkt_sb[0:DH, :], wkt[:, :])
        nc.scalar.dma_start(wkt_sb[DH : 2 * DH, :], wkt[:, :])
        wo_sb = cpool.tile([P, DIM], F16, name="wo_sb")
        nc.gpsimd.dma_start(wo_sb, wo[:, :])

        # ---- persistent SBUF tiles ----
        sqT = qp.tile([P, N], F16, tag="q", name="sqT")
        skT = sb.tile([P, N], F16, name="skT")
        rqT = sb.tile([P, N], F16, name="rqT")
        rv16 = sb.tile([P, NT, P], F16, name="rv16")
        ET0 = sb.tile([P, NT, N], F16, name="ET0")
        ET1 = sb.tile([P, NT - 4, N], F16, name="ET1")
        uT16 = [sb.tile([P, N], F16, name=f"uT16_{h}") for h in range(HPC)]
        un = [sb.tile([P, NT, R, DH], F16, name=f"un{h}") for h in range(HPC)]
        rqw = [sb.tile([P, NT, DH], F16, name=f"rqw{h}") for h in range(HPC)]
        uc2 = sb.tile([P, NT, P], F16, name="uc2")
        gdt = [sb.tile([P, NT], F32, name=f"gdt{h}") for h in range(HPC)]
        zit = [sb.tile([P, NT], F32, name=f"zit{h}") for h in range(HPC)]
        state = {"ET1d": None, "uf2": None}

        def et(h, jt):
            if h == 0:
                return ET0, jt
            if jt < NT - 4:
                return ET1, jt
            return state["ET1d"], jt - (NT - 4)

        # ---- q/k projections: kc-outer so matmuls start on the first
        # arriving x chunk ----
        def proj_icp(wsb, dst, icp, ticks=0):
            pp = pst.tile([P, 1024], F32, tag="st", name="pp")
            for kc in range(KC):
                for half in range(2):
                    i0 = icp * 1024 + half * 512
                    nc.tensor.matmul(
                        pp[:, half * 512 : (half + 1) * 512],
                        wsb[:, kc, :],
                        xT[:, kc, i0 : i0 + 512],
                        start=(kc == 0),
                        stop=(kc == KC - 1),
                        skip_group_check=True,
                    )
                tick(ticks)
            dst_ap = dst[:, icp * 1024 : (icp + 1) * 1024]
            if icp == 0:
                nc.scalar.copy(out=dst_ap, in_=pp)
            else:
                # keep the ACT queue free for the first exps
                nc.vector.tensor_copy(out=dst_ap, in_=pp)

        proj_icp(wq_sb, sqT, 0, ticks=6)
        proj_icp(wk_sb, skT, 0, ticks=6)
        proj_icp(wq_sb, sqT, 1)
        proj_icp(wk_sb, skT, 1)

        def rv_chunk(ic):
            pv = pwk.tile([P, 512], F32, tag="wk", name="pv")
            for kc in range(KC):
                nc.tensor.matmul(
                    pv,
                    wv_sb[:, kc, :],
                    xT[:, kc, ic * 512 : (ic + 1) * 512],
                    start=(kc == 0),
                    stop=(kc == KC - 1),
                )
            rvT_c = hd.tile([P, 512], F16, tag="rvT", name="rvT_c")
            nc.vector.tensor_copy(out=rvT_c, in_=pv)
            pt4 = pwk.tile([P, 512], F16, tag="wk", name="pt4")
            for t in range(4):
                nc.tensor.transpose(
                    pt4[:, t * P : (t + 1) * P], rvT_c[:, t * P : (t + 1) * P],
                    ident,
                )
            nc.vector.tensor_copy(out=rv16[:, ic * 4 : (ic + 1) * 4, :], in_=pt4)

        def rq_chunk(icp):
            for half in range(2):
                i0 = icp * 1024 + half * 512
                pp = pwk.tile([P, 512], F32, tag="wk", name="prq")
                for kc in range(KC):
                    nc.tensor.matmul(
                        pp,
                        wqr_sb[:, kc, :],
                        xT[:, kc, i0 : i0 + 512],
                        start=(kc == 0),
                        stop=(kc == KC - 1),
                    )
                nc.vector.tensor_copy(out=rqT[:, i0 : i0 + 512], in_=pp)

        rv_chunk(0)  # av(h0, jt=0) needs rv16[0..3] early

        def rqw_four(h, b4):
            """4 rqw matmuls into one PSUM tile, one batched copy."""
            hs = slice(h * DH, (h + 1) * DH)
            pq = pwk.tile([P, 4 * DH], F32, tag="wk", name="pq")
            for t in range(4):
                it = b4 * 4 + t
                nc.tensor.matmul(
                    pq[:, t * DH : (t + 1) * DH],
                    rqT[hs, it * P : (it + 1) * P],
                    wkt_sb[hs, :],
                    start=True,
                    stop=True,
                    skip_group_check=True,
                )
            nc.vector.tensor_copy(out=rqw[h][:, b4 * 4 : (b4 + 1) * 4, :], in_=pq)

        def stage2_pre(h, hf):
            """sims + gating logits for half hf (8 it's); batched DVE."""
            its = slice(8 * hf, 8 * hf + 8)
            sims = []
            for r in range(R):
                prod = hd.tile([P, 8, DH], F16, tag="prod", name="prod")
                nc.vector.tensor_tensor(
                    prod, un[h][:, its, r, :], rqw[h][:, its, :], ALU.mult
                )
                s_ = hd.tile([P, 8], F32, tag=f"sims{r}", name=f"sims{r}")
                nc.vector.tensor_reduce(
                    s_, prod, axis=mybir.AxisListType.X, op=ALU.add
                )
                sims.append(s_)
            gds = gdt[h][:, its]
            nc.vector.tensor_tensor(gds, sims[0], sims[1], ALU.subtract)
            nc.vector.tensor_tensor(gds, gds, zit[h][:, its], ALU.mult)

        def stage2_post(h, hf):
            """Gating exp + uc writes for half hf (8 it's)."""
            its = slice(8 * hf, 8 * hf + 8)
            p0 = hd.tile([P, 8], F32, tag="p0", name="p0")
            nc.scalar.activation(p0, gdt[h][:, its], AF.Exp)
            w_ = hd.tile([P, 8], F32, tag="w_", name="w_")
            nc.vector.tensor_scalar_add(w_, p0, 1.0)
            nc.vector.reciprocal(w_, w_)
            nc.vector.tensor_tensor(w_, w_, zit[h][:, its], ALU.mult)
            a0z = hd.tile([P, 8], F32, tag="a0z", name="a0z")
            nc.vector.tensor_tensor(a0z, w_, p0, ALU.mult)
            ucs = uc2[:, its, h * DH : (h + 1) * DH]
            nc.vector.tensor_tensor(
                ucs, un[h][:, its, 0, :],
                a0z[:, :, None].to_broadcast((P, 8, DH)), ALU.mult,
            )
            t1 = hd.tile([P, 8, DH], F16, tag="t1", name="t1")
            nc.vector.tensor_tensor(
                t1, un[h][:, its, 1, :],
                w_[:, :, None].to_broadcast((P, 8, DH)), ALU.mult,
            )
            nc.vector.tensor_tensor(ucs, ucs, t1, ALU.add)

        def avt_pass(h, c):
            pu = pwk.tile([P, 512], F32, tag="wk", name=f"pu{h}{c}")
            for jt in range(NT):
                def mm(jt=jt, c=c, pu=pu):
                    t_, lj = et(h, jt)
                    nc.tensor.matmul(
                        pu,
                        rv16[:, jt, :],
                        t_[:, lj, c * 512 : (c + 1) * 512],
                        start=(jt == 0),
                        stop=(jt == NT - 1),
                        skip_group_check=True,
                    )
                yield mm
            def cp(c=c, pu=pu):
                nc.vector.tensor_copy(
                    out=uT16[h][:, c * 512 : (c + 1) * 512], in_=pu
                )
            yield cp

        def unT_four(h, b4, on_scalar=False):
            """4 u transposes into one PSUM tile + one batched copy."""
            pt2 = pwk.tile([P, 512], F16, tag="wk", name="pt2")
            for t in range(4):
                it = b4 * 4 + t
                def tr(t=t, it=it, pt2=pt2):
                    nc.tensor.transpose(
                        pt2[:, t * P : (t + 1) * P],
                        uT16[h][:, it * P : (it + 1) * P], ident,
                    )
                yield tr
            def cp(pt2=pt2, b4=b4):
                dst = un[h][:, b4 * 4 : (b4 + 1) * 4]
                if on_scalar:
                    nc.scalar.copy(out=dst, in_=pt2)
                else:
                    nc.vector.tensor_copy(out=dst, in_=pt2)
            yield cp

        def z_adds(h, q, eng):
            """Level-1 pair adds (in place) for i-quarter q on engine eng."""
            cs = slice(q * 512, (q + 1) * 512)
            for t in range(8):
                def add(t=t, cs=cs, eng=eng):
                    ta, a = et(h, 2 * t)
                    tb, b_ = et(h, 2 * t + 1)
                    eng.tensor_tensor(
                        ta[:, a, cs], ta[:, a, cs], tb[:, b_, cs], ALU.add
                    )
                yield add

        def z_extract(h, q, pz):
            """Z row [1,512] -> zit columns via 4 tiny PE transposes."""
            zrow16 = hd.tile([1, 512], F16, tag="zrow", name="zrow16")
            def zcp(pz=pz, zrow16=zrow16):
                nc.vector.tensor_copy(out=zrow16, in_=pz)
            yield zcp
            pzt = pwk.tile([P, 8], F16, tag="wk", name="pzt")
            for t in range(4):
                def ztr(t=t, pzt=pzt, zrow16=zrow16):
                    nc.tensor.transpose(
                        pzt[:, 2 * t : 2 * t + 1],
                        zrow16[:, t * P : (t + 1) * P],
                        ident[0:1, 0:1],
                    )
                yield ztr
            def zfin(pzt=pzt, h=h, q=q):
                nc.vector.tensor_copy(
                    out=zit[h][:, q * 4 : (q + 1) * 4], in_=pzt[:, 0:8:2]
                )
            yield zfin

        def z_mms(h, q):
            """8 ones-matmuls over pair-summed tiles + extraction."""
            cs = slice(q * 512, (q + 1) * 512)
            pz = pwk.tile([1, 512], F32, tag="wk", name=f"pz{h}{q}")
            for g in range(8):
                def mm(g=g, pz=pz, cs=cs):
                    t_, lj = et(h, 2 * g)
                    nc.tensor.matmul(
                        pz,
                        ones16,
                        t_[:, lj, cs],
                        start=(g == 0),
                        stop=(g == 7),
                    )
                yield mm
            yield from z_extract(h, q, pz)

        def z_pass16(h, q):
            """16 ones-matmuls over the raw exp tiles (no DVE adds) +
            extraction; used in the tail where the PE has slack."""
            cs = slice(q * 512, (q + 1) * 512)
            pz = pwk.tile([1, 512], F32, tag="wk", name=f"pzp{h}{q}")
            for jt in range(NT):
                def mm(jt=jt, pz=pz, cs=cs):
                    t_, lj = et(h, jt)
                    nc.tensor.matmul(
                        pz,
                        ones16,
                        t_[:, lj, cs],
                        start=(jt == 0),
                        stop=(jt == NT - 1),
                    )
                yield mm
            yield from z_extract(h, q, pz)

        def z_recip(h, hf):
            nc.vector.reciprocal(
                zit[h][:, 8 * hf : 8 * hf + 8], zit[h][:, 8 * hf : 8 * hf + 8]
            )

        def paired(g1, g2):
            """Interleave two op generators (they use the two wk buffers)."""
            while True:
                a = next(g1, None)
                b = next(g2, None)
                if a is None and b is None:
                    return
                if a is not None:
                    yield a
                if b is not None:
                    yield b

        def trail0():
            """Head-0 trailing work, drained inside head-1's stream.
            DVE adds lead so the vector FIFO never blocks behind PE work;
            avt (PE) runs concurrently; stage-2 lands late."""
            # q0/q1 pair adds: av01 already consumed those columns
            yield from z_adds(0, 0, nc.vector)
            yield from z_adds(0, 1, nc.vector)
            # AV c2/c3 read the raw exp tiles (cols 1024:2048)
            yield from paired(avt_pass(0, 2), avt_pass(0, 3))
            yield from z_mms(0, 0)
            yield from z_mms(0, 1)
            yield lambda: z_recip(0, 0)
            yield from z_adds(0, 2, nc.vector)
            yield from z_adds(0, 3, nc.vector)
            yield from unT_four(0, 0)
            yield from unT_four(0, 1)
            yield from z_mms(0, 2)
            yield from z_mms(0, 3)
            yield lambda: z_recip(0, 1)
            yield from unT_four(0, 2)
            yield from unT_four(0, 3)
            yield lambda: stage2_pre(0, 0)
            yield lambda: stage2_pre(0, 1)
            yield lambda: stage2_post(0, 0)
            yield lambda: stage2_post(0, 1)

        def out_a_four(b4):
            """4 uc2 transposes -> uf2 with one batched copy."""
            pf = pwk.tile([P, 512], F16, tag="wk", name="pf")
            for t in range(4):
                it = b4 * 4 + t
                nc.tensor.transpose(
                    pf[:, t * P : (t + 1) * P], uc2[:, it], ident
                )
            nc.scalar.copy(
                out=state["uf2"][:, b4 * 512 : (b4 + 1) * 512], in_=pf
            )

        def out_b(it):
            """Out-projection matmul + ysb copy (alternating engines) + DMA."""
            py = pst.tile([P, DIM], F32, tag="st", name="py")
            nc.tensor.matmul(
                py,
                state["uf2"][:, it * P : (it + 1) * P],
                wo_sb,
                start=True,
                stop=True,
            )
            ysb = hd.tile([P, DIM], F16, tag="ysb", name="ysb")
            if it % 2 == 0:
                nc.scalar.copy(out=ysb, in_=py)
            else:
                nc.vector.tensor_copy(out=ysb, in_=py)
            q = nc.sync if it % 2 == 0 else nc.gpsimd
            q.dma_start(y[it * P : (it + 1) * P, :], ysb)

        # -------- main per-head jt-loops --------
        trail = None
        for h in range(HPC):
            hs = slice(h * DH, (h + 1) * DH)
            if h == 1:
                trail = trail0()
                state["ET1d"] = xp.tile([P, 4, N], F16, tag="x", name="ET1d")
                state["uf2"] = qp.tile([P, N], F16, tag="q", name="uf2")
            av01 = pav.tile([P, 1024], F32, tag="av", name=f"av01_{h}")
            for jt in range(NT):
                th, lj = et(h, jt)
                for icp in range(2):
                    st = pst.tile([P, 1024], F32, tag="st", name="st")
                    for half in range(2):
                        i0 = icp * 1024 + half * 512
                        nc.tensor.matmul(
                            st[:, half * 512 : (half + 1) * 512],
                            skT[hs, jt * P : (jt + 1) * P],
                            sqT[hs, i0 : i0 + 512],
                            start=True,
                            stop=True,
                            skip_group_check=True,
                        )
                    nc.scalar.activation(
                        th[:, lj, icp * 1024 : (icp + 1) * 1024], st, AF.Exp
                    )
                if jt >= 2:
                    ajt = jt - 2
                    ta, la = et(h, ajt)
                    for c in range(2):
                        nc.tensor.matmul(
                            av01[:, c * 512 : (c + 1) * 512],
                            rv16[:, ajt, :],
                            ta[:, la, c * 512 : (c + 1) * 512],
                            start=(ajt == 0),
                            stop=False,
                            skip_group_check=True,
                        )
                if h == 0:
                    if jt in (1, 3, 5):
                        rv_chunk((jt + 1) // 2)
                    elif jt in (7, 9):
                        rq_chunk((jt - 7) // 2)
                    elif 10 <= jt <= 13:
                        b0 = (jt - 10) * 2
                        rqw_four(b0 // 4, b0 % 4)
                        rqw_four((b0 + 1) // 4, (b0 + 1) % 4)
                        tick(4)
                    else:
                        # sparse jt: keep the PE-activity window busy so the
                        # HAM clock-gate doesn't flap mid-stream
                        tick(8)
                else:
                    ndrain = 12 if jt < 10 else 5
                    for _ in range(ndrain):
                        op = next(trail, None)
                        if op is None:
                            break
                        op()

            def av01_tail(h=h, av01=av01):
                for ajt in (NT - 2, NT - 1):
                    ta, la = et(h, ajt)
                    for c in range(2):
                        nc.tensor.matmul(
                            av01[:, c * 512 : (c + 1) * 512],
                            rv16[:, ajt, :],
                            ta[:, la, c * 512 : (c + 1) * 512],
                            start=False,
                            stop=(ajt == NT - 1),
                            skip_group_check=True,
                        )
                if h == 1:
                    nc.scalar.copy(out=uT16[h][:, 0:1024], in_=av01)
                else:
                    nc.vector.tensor_copy(out=uT16[h][:, 0:1024], in_=av01)

            if h == 0:
                av01_tail()
            else:
                for op in trail:
                    op()
                # avt c2/c3 only read exp cols 1024:2048 and jt<=13 early, so
                # the PE stays busy while the last two exps finish; av01's
                # tail (which reads cols 0:1024 of jt14/15) lands behind it.
                for op in paired(avt_pass(1, 2), avt_pass(1, 3)):
                    op()
                av01_tail()

        # -------- tail: head-1 trailing work pipelined with the
        # out-projection; PE-dense ordering to avoid HAM re-throttle --------
        for op in unT_four(1, 0, on_scalar=True):
            op()
        for op in unT_four(1, 1, on_scalar=True):
            op()
        for op in paired(z_pass16(1, 0), z_pass16(1, 1)):
            op()
        z_recip(1, 0)
        stage2_pre(1, 0)
        for op in unT_four(1, 2, on_scalar=True):
            op()
        for op in unT_four(1, 3, on_scalar=True):
            op()
        stage2_post(1, 0)
        for op in paired(z_pass16(1, 2), z_pass16(1, 3)):
            op()
        z_recip(1, 1)
        tick(4)
        out_a_four(0)
        out_a_four(1)
        stage2_pre(1, 1)
        tick(4)
        for it in range(0, 8):
            out_b(it)
            tick(2)
        stage2_post(1, 1)
        tick(4)
        out_a_four(2)
        out_a_four(3)
        for it in range(8, 16):
            out_b(it)
            tick(2)


def build_program():
    nc = bacc.Bacc(None, target_bir_lowering=False)
    xt = nc.declare_dram_parameter("xt", [DIM, N], F16, isOutput=False)
    wq = nc.declare_dram_parameter("wq", [DIM, P], F16, isOutput=False)
    wk = nc.declare_dram_parameter("wk", [DIM, P], F16, isOutput=False)
    wv = nc.declare_dram_parameter("wv", [DIM, P], F16, isOutput=False)
    wqr = nc.declare_dram_parameter("wqr", [DIM, P], F16, isOutput=False)
    wkt = nc.declare_dram_parameter("wkt", [DH, DH], F16, isOutput=False)
    wo = nc.declare_dram_parameter("wo", [P, DIM], F16, isOutput=False)
    y = nc.declare_dram_parameter("y", [N, DIM], F16, isOutput=True)

    with tile.TileContext(nc) as tc:
        _emit(tc, xt, wq, wk, wv, wqr, wkt, wo, y)
    nc.compile()
    return nc


_NC_CACHE = None


def _get_program():
    global _NC_CACHE
    if _NC_CACHE is None:
        _NC_CACHE = build_program()
    return _NC_CACHE


def make_in_maps(inputs):
    x = np.asarray(inputs["x"], dtype=np.float32)
    wq_s = np.asarray(inputs["wq_s"], dtype=np.float32)
    wk_s = np.asarray(inputs["wk_s"], dtype=np.float32)
    wv_r = np.asarray(inputs["wv_r"], dtype=np.float32)
    wq_r = np.asarray(inputs["wq_r"], dtype=np.float32)
    wk_ret = np.asarray(inputs["wk_ret"], dtype=np.float32)
    w_out = np.asarray(inputs["w_out"], dtype=np.float32)
    scale = np.float32(DH**-0.5)

    f16 = np.float16
    in_maps = []
    for c in range(NCORES):
        b, hp = divmod(c, NCORES // B)
        cols = slice(hp * P, (hp + 1) * P)
        in_maps.append(
            {
                "xt": np.ascontiguousarray(x[b].T).astype(f16),
                "wq": (np.ascontiguousarray(wq_s[:, cols]) * scale).astype(f16),
                "wk": np.ascontiguousarray(wk_s[:, cols]).astype(f16),
                "wv": wv_r.astype(f16),
                "wqr": (np.ascontiguousarray(wq_r[:, cols]) * scale).astype(f16),
                "wkt": np.ascontiguousarray(wk_ret.T).astype(f16),
                "wo": np.ascontiguousarray(w_out[hp * P : (hp + 1) * P, :]).astype(f16),
            }
        )
    return in_maps


def run(inputs, trace=False, **kw):
    res = run_bass_kernel_spmd(
        _get_program(), make_in_maps(inputs), list(range(NCORES)), trace=trace, **kw
    )
    out = np.zeros((B, N, DIM), np.float32)
    for c in range(NCORES):
        out[c // (NCORES // B)] += np.asarray(res.results[c]["y"], np.float32)
    return out, res


def kernel(**inputs):
    out, _ = run(inputs)
    return out


# revision 16
# speedup vs baseline: 1.0803x; 1.0702x over previous
"""Compositional attention kernel for Trainium2, 8-core SPMD.

Sharding: core c handles batch b = c // 4 and search-heads {2*(c%4), 2*(c%4)+1}
end-to-end (tensor-parallel over the S=8 search heads x data-parallel over
B=2).  Each core computes a partial y = out_heads @ w_out[head_rows] for its
batch in fp16; the host sums the 4 partials per batch in fp32.

All matmuls run in fp16 with fp32 PSUM accumulation.  Schedule notes:
- ~32 warm-up matmuls (ident@ident) run during the input-DMA wait so the PE
  HAM clock-gate opens (1.2->2.4 GHz) before the first real matmul.
- Input DMAs are spread over 4 queues (sync/scalar/vector/gpsimd).
- Z is computed with an in-place pair-tree (8 DVE adds + 8 ones-matmuls per
  i-quarter) for BOTH heads; the Z row [1,512] is turned into the per-i
  partition layout with 4 tiny PE transposes (no DRAM round-trip).
- DVE work is batched 4-wide (rv16/un/rqw/uf2 copies) and stage-2 runs at
  half-head width to amortize per-instruction overhead.
- Head-0's trailing work (AV c2/c3, Z tree, un transposes, stage-2) drains
  11 ops/jt inside head-1's score/exp stream; head-1's trail is pipelined in
  the tail with the out-projection so the PE never idles long enough to
  re-throttle.
"""

import sys

import numpy as np

for _p in ("/opt/trn_rl_repo", "/root/.axon_site/_ro/trn_rl_repo"):
    if _p not in sys.path:
        sys.path.append(_p)

import concourse.bass as bass  # noqa: F401
import concourse.mybir as mybir
import concourse.tile as tile
from concourse import bacc
from concourse.bass_utils import run_bass_kernel_spmd
from concourse.masks import make_identity

S, R, DH = 8, 2, 64
B, N, DIM = 2, 2048, 512
P = 128
NT = N // P  # 16 j-tiles
KC = DIM // P  # 4 contraction chunks of x
NCORES = 8
HPC = 2  # heads per core

F32 = mybir.dt.float32
F16 = mybir.dt.float16
AF = mybir.ActivationFunctionType
ALU = mybir.AluOpType

N_WARM = 24


def _emit(tc, xt, wq, wk, wv, wqr, wkt, wo, y):
    from contextlib import ExitStack

    nc = tc.nc
    with ExitStack() as ctx:
        cpool = ctx.enter_context(tc.tile_pool(name="const", bufs=1))
        xp = ctx.enter_context(tc.tile_pool(name="xp", bufs=1))
        qp = ctx.enter_context(tc.tile_pool(name="qp", bufs=1))
        sb = ctx.enter_context(tc.tile_pool(name="sb", bufs=1))
        hd = ctx.enter_context(tc.tile_pool(name="hd", bufs=2))
        yp = ctx.enter_context(tc.tile_pool(name="yp", bufs=4))
        # PSUM: st 2x[128,1024]f32 = 4 banks, av 1x[128,1024]f32 = 2 banks,
        # wk 2x[<=2KB] = 2 banks  -> 8 banks exactly.
        pst = ctx.enter_context(tc.tile_pool(name="pst", bufs=2, space="PSUM"))
        pav = ctx.enter_context(tc.tile_pool(name="pav", bufs=1, space="PSUM"))
        pwk = ctx.enter_context(tc.tile_pool(name="pwk", bufs=2, space="PSUM"))

        # ---- constants ----
        ones16 = cpool.tile([P, 1], F16, name="ones16")
        nc.gpsimd.memset(ones16, 1.0)
        ident = cpool.tile([P, P], F16, name="ident")
        make_identity(nc, ident)

        # ---- PE warm-up: keep the PE busy during the DMA wait so the HAM
        # clock-gate opens before the first projection matmul.  Tiny
        # ones-matmuls start as soon as the memset lands; ident matmuls
        # (128-col streams) follow once make_identity finishes. ----
        pw1 = pav.tile([1, 1], F32, tag="av", name="pw1")
        for _ in range(24):
            nc.tensor.matmul(
                pw1, ones16, ones16, start=True, stop=True,
                skip_group_check=True,
            )
        pwarm = pav.tile([P, P], F32, tag="av", name="pwarm")
        for _ in range(N_WARM):
            nc.tensor.matmul(
                pwarm, ident, ident, start=True, stop=True,
                skip_group_check=True,
            )

        def tick(n):
            """Standalone ident weight-loads: PE-array activity with no PSUM
            hazards.  The PE's LDWEIGHTS pull-ahead lets these run during
            stalls of in-flight matmuls, keeping the HAM clock-gate open
            through DMA waits and ACT-paced stretches."""
            for _ in range(n):
                nc.tensor.ldweights(weights=ident)

        # x^T in half-chunks spread over the 3 DMA-capable queues: the
        # i 0:1024 halves land first so the first projection/score/exp
        # chain starts as early as possible.
        xT = xp.tile([P, KC, N], F16, tag="x", name="xT")
        wq_sb = cpool.tile([P, KC, P], F16, name="wq_sb")
        nc.scalar.dma_start(wq_sb, wq.rearrange("(kc p) m -> p kc m", p=P))
        qs = [nc.sync, nc.gpsimd, nc.sync, nc.gpsimd]
        for kc in range(KC):
            qs[kc].dma_start(
                xT[:, kc, 0:1024], xt[kc * P : (kc + 1) * P, 0:1024]
            )
        wk_sb = cpool.tile([P, KC, P], F16, name="wk_sb")
        nc.scalar.dma_start(wk_sb, wk.rearrange("(kc p) m -> p kc m", p=P))
        for kc in range(KC):
            qs[kc].dma_start(
                xT[:, kc, 1024:2048], xt[kc * P : (kc + 1) * P, 1024:2048]
            )
        wv_sb = cpool.tile([P, KC, P], F16, name="wv_sb")
        nc.gpsimd.dma_start(wv_sb, wv.rearrange("(kc p) m -> p kc m", p=P))
        wqr_sb = cpool.tile([P, KC, P], F16, name="wqr_sb")
        nc.sync.dma_start(wqr_sb, wqr.rearrange("(kc p) m -> p kc m", p=P))
        wkt_sb = cpool.tile([2 * DH, DH], F16, name="wkt_sb")
        nc.scalar.dma_start(wkt_sb[0:DH, :], wkt[:, :])
        nc.scalar.dma_start(wkt_sb[DH : 2 * DH, :], wkt[:, :])
        wo_sb = cpool.tile([P, DIM], F16, name="wo_sb")
        nc.gpsimd.dma_start(wo_sb, wo[:, :])

        # ---- persistent SBUF tiles ----
        sqT = qp.tile([P, N], F16, tag="q", name="sqT")
        skT = sb.tile([P, N], F16, name="skT")
        rqT = sb.tile([P, N], F16, name="rqT")
        rv16 = sb.tile([P, NT, P], F16, name="rv16")
        ET0 = sb.tile([P, NT, N], F16, name="ET0")
        ET1 = sb.tile([P, NT - 4, N], F16, name="ET1")
        uT16 = [sb.tile([P, N], F16, name=f"uT16_{h}") for h in range(HPC)]
        un = [sb.tile([P, NT, R, DH], F16, name=f"un{h}") for h in range(HPC)]
        rqw = [sb.tile([P, NT, DH], F16, name=f"rqw{h}") for h in range(HPC)]
        uc2 = sb.tile([P, NT, P], F16, name="uc2")
        gdt = [sb.tile([P, NT], F32, name=f"gdt{h}") for h in range(HPC)]
        zit = [sb.tile([P, NT], F32, name=f"zit{h}") for h in range(HPC)]
        state = {"ET1d": None, "uf2": None}

        def et(h, jt):
            if h == 0:
                return ET0, jt
            if jt < NT - 4:
                return ET1, jt
            return state["ET1d"], jt - (NT - 4)

        # ---- q/k projections: kc-outer so matmuls start on the first
        # arriving x chunk ----
        def proj_icp(wsb, dst, icp, ticks=0):
            pp = pst.tile([P, 1024], F32, tag="st", name="pp")
            for kc in range(KC):
                for half in range(2):
                    i0 = icp * 1024 + half * 512
                    nc.tensor.matmul(
                        pp[:, half * 512 : (half + 1) * 512],
                        wsb[:, kc, :],
                        xT[:, kc, i0 : i0 + 512],
                        start=(kc == 0),
                        stop=(kc == KC - 1),
                        skip_group_check=True,
                    )
                tick(ticks)
            dst_ap = dst[:, icp * 1024 : (icp + 1) * 1024]
            if icp == 0:
                nc.scalar.copy(out=dst_ap, in_=pp)
            else:
                # keep the ACT queue free for the first exps
                nc.vector.tensor_copy(out=dst_ap, in_=pp)

        proj_icp(wq_sb, sqT, 0, ticks=6)
        proj_icp(wk_sb, skT, 0, ticks=6)
        proj_icp(wq_sb, sqT, 1)
        proj_icp(wk_sb, skT, 1)

        def rv_chunk(ic):
            pv = pwk.tile([P, 512], F32, tag="wk", name="pv")
            for kc in range(KC):
                nc.tensor.matmul(
                    pv,
                    wv_sb[:, kc, :],
                    xT[:, kc, ic * 512 : (ic + 1) * 512],
                    start=(kc == 0),
                    stop=(kc == KC - 1),
                )
            rvT_c = hd.tile([P, 512], F16, tag="rvT", name="rvT_c")
            nc.vector.tensor_copy(out=rvT_c, in_=pv)
            pt4 = pwk.tile([P, 512], F16, tag="wk", name="pt4")
            for t in range(4):
                nc.tensor.transpose(
                    pt4[:, t * P : (t + 1) * P], rvT_c[:, t * P : (t + 1) * P],
                    ident,
                )
            nc.vector.tensor_copy(out=rv16[:, ic * 4 : (ic + 1) * 4, :], in_=pt4)

        def rq_chunk(icp):
            for half in range(2):
                i0 = icp * 1024 + half * 512
                pp = pwk.tile([P, 512], F32, tag="wk", name="prq")
                for kc in range(KC):
                    nc.tensor.matmul(
                        pp,
                        wqr_sb[:, kc, :],
                        xT[:, kc, i0 : i0 + 512],
                        start=(kc == 0),
                        stop=(kc == KC - 1),
                    )
                nc.vector.tensor_copy(out=rqT[:, i0 : i0 + 512], in_=pp)

        rv_chunk(0)  # av(h0, jt=0) needs rv16[0..3] early

        def rqw_four(h, b4):
            """4 rqw matmuls into one PSUM tile, one batched copy."""
            hs = slice(h * DH, (h + 1) * DH)
            pq = pwk.tile([P, 4 * DH], F32, tag="wk", name="pq")
            for t in range(4):
                it = b4 * 4 + t
                nc.tensor.matmul(
                    pq[:, t * DH : (t + 1) * DH],
                    rqT[hs, it * P : (it + 1) * P],
                    wkt_sb[hs, :],
                    start=True,
                    stop=True,
                    skip_group_check=True,
                )
            nc.vector.tensor_copy(out=rqw[h][:, b4 * 4 : (b4 + 1) * 4, :], in_=pq)

        def stage2_pre(h, hf):
            """sims + gating logits for half hf (8 it's); batched DVE."""
            its = slice(8 * hf, 8 * hf + 8)
            sims = []
            for r in range(R):
                prod = hd.tile([P, 8, DH], F16, tag="prod", name="prod")
                nc.vector.tensor_tensor(
                    prod, un[h][:, its, r, :], rqw[h][:, its, :], ALU.mult
                )
                s_ = hd.tile([P, 8], F32, tag=f"sims{r}", name=f"sims{r}")
                nc.vector.tensor_reduce(
                    s_, prod, axis=mybir.AxisListType.X, op=ALU.add
                )
                sims.append(s_)
            gds = gdt[h][:, its]
            nc.vector.tensor_tensor(gds, sims[0], sims[1], ALU.subtract)
            nc.vector.tensor_tensor(gds, gds, zit[h][:, its], ALU.mult)

        def stage2_post(h, hf):
            """Gating exp + uc writes for half hf (8 it's)."""
            its = slice(8 * hf, 8 * hf + 8)
            p0 = hd.tile([P, 8], F32, tag="p0", name="p0")
            nc.scalar.activation(p0, gdt[h][:, its], AF.Exp)
            w_ = hd.tile([P, 8], F32, tag="w_", name="w_")
            nc.vector.tensor_scalar_add(w_, p0, 1.0)
            nc.vector.reciprocal(w_, w_)
            nc.vector.tensor_tensor(w_, w_, zit[h][:, its], ALU.mult)
            a0z = hd.tile([P, 8], F32, tag="a0z", name="a0z")
            nc.vector.tensor_tensor(a0z, w_, p0, ALU.mult)
            ucs = uc2[:, its, h * DH : (h + 1) * DH]
            nc.vector.tensor_tensor(
                ucs, un[h][:, its, 0, :],
                a0z[:, :, None].to_broadcast((P, 8, DH)), ALU.mult,
            )
            t1 = hd.tile([P, 8, DH], F16, tag="t1", name="t1")
            nc.vector.tensor_tensor(
                t1, un[h][:, its, 1, :],
                w_[:, :, None].to_broadcast((P, 8, DH)), ALU.mult,
            )
            nc.vector.tensor_tensor(ucs, ucs, t1, ALU.add)

        def avt_pass(h, c):
            pu = pwk.tile([P, 512], F32, tag="wk", name=f"pu{h}{c}")
            for jt in range(NT):
                def mm(jt=jt, c=c, pu=pu):
                    t_, lj = et(h, jt)
                    nc.tensor.matmul(
                        pu,
                        rv16[:, jt, :],
                        t_[:, lj, c * 512 : (c + 1) * 512],
                        start=(jt == 0),
                        stop=(jt == NT - 1),
                        skip_group_check=True,
                    )
                yield mm
            def cp(c=c, pu=pu):
                nc.vector.tensor_copy(
                    out=uT16[h][:, c * 512 : (c + 1) * 512], in_=pu
                )
            yield cp

        def unT_four(h, b4, on_scalar=False):
            """4 u transposes into one PSUM tile + one batched copy."""
            pt2 = pwk.tile([P, 512], F16, tag="wk", name="pt2")
            for t in range(4):
                it = b4 * 4 + t
                def tr(t=t, it=it, pt2=pt2):
                    nc.tensor.transpose(
                        pt2[:, t * P : (t + 1) * P],
                        uT16[h][:, it * P : (it + 1) * P], ident,
                    )
                yield tr
            def cp(pt2=pt2, b4=b4):
                dst = un[h][:, b4 * 4 : (b4 + 1) * 4]
                if on_scalar:
                    nc.scalar.copy(out=dst, in_=pt2)
                else:
                    nc.vector.tensor_copy(out=dst, in_=pt2)
            yield cp

        def z_adds(h, q, eng):
            """Level-1 pair adds (in place) for i-quarter q on engine eng."""
            cs = slice(q * 512, (q + 1) * 512)
            for t in range(8):
                def add(t=t, cs=cs, eng=eng):
                    ta, a = et(h, 2 * t)
                    tb, b_ = et(h, 2 * t + 1)
                    eng.tensor_tensor(
                        ta[:, a, cs], ta[:, a, cs], tb[:, b_, cs], ALU.add
                    )
                yield add

        def z_extract(h, q, pz):
            """Z row [1,512] -> zit columns via 4 tiny PE transposes."""
            zrow16 = hd.tile([1, 512], F16, tag="zrow", name="zrow16")
            def zcp(pz=pz, zrow16=zrow16):
                nc.vector.tensor_copy(out=zrow16, in_=pz)
            yield zcp
            pzt = pwk.tile([P, 8], F16, tag="wk", name="pzt")
            for t in range(4):
                def ztr(t=t, pzt=pzt, zrow16=zrow16):
                    nc.tensor.transpose(
                        pzt[:, 2 * t : 2 * t + 1],
                        zrow16[:, t * P : (t + 1) * P],
                        ident[0:1, 0:1],
                    )
                yield ztr
            def zfin(pzt=pzt, h=h, q=q):
                nc.vector.tensor_copy(
                    out=zit[h][:, q * 4 : (q + 1) * 4], in_=pzt[:, 0:8:2]
                )
            yield zfin

        def z_mms(h, q):
            """8 ones-matmuls over pair-summed tiles + extraction."""
            cs = slice(q * 512, (q + 1) * 512)
            pz = pwk.tile([1, 512], F32, tag="wk", name=f"pz{h}{q}")
            for g in range(8):
                def mm(g=g, pz=pz, cs=cs):
                    t_, lj = et(h, 2 * g)
                    nc.tensor.matmul(
                        pz,
                        ones16,
                        t_[:, lj, cs],
                        start=(g == 0),
                        stop=(g == 7),
                    )
                yield mm
            yield from z_extract(h, q, pz)

        def z_pass16(h, q):
            """16 ones-matmuls over the raw exp tiles (no DVE adds) +
            extraction; used in the tail where the PE has slack."""
            cs = slice(q * 512, (q + 1) * 512)
            pz = pwk.tile([1, 512], F32, tag="wk", name=f"pzp{h}{q}")
            for jt in range(NT):
                def mm(jt=jt, pz=pz, cs=cs):
                    t_, lj = et(h, jt)
                    nc.tensor.matmul(
                        pz,
                        ones16,
                        t_[:, lj, cs],
                        start=(jt == 0),
                        stop=(jt == NT - 1),
                    )
                yield mm
            yield from z_extract(h, q, pz)

        def z_recip(h, hf):
            nc.vector.reciprocal(
                zit[h][:, 8 * hf : 8 * hf + 8], zit[h][:, 8 * hf : 8 * hf + 8]
            )

        def paired(g1, g2):
            """Interleave two op generators (they use the two wk buffers)."""
            while True:
                a = next(g1, None)
                b = next(g2, None)
                if a is None and b is None:
                    return
                if a is not None:
                    yield a
                if b is not None:
                    yield b

        def trail0():
            """Head-0 trailing work, drained inside head-1's stream.
            DVE adds lead so the vector FIFO never blocks behind PE work;
            avt (PE) runs concurrently; stage-2 lands late."""
            # q0/q1 pair adds: av01 already consumed those columns
            yield from z_adds(0, 0, nc.vector)
            yield from z_adds(0, 1, nc.vector)
            # AV c2/c3 read the raw exp tiles (cols 1024:2048)
            yield from paired(avt_pass(0, 2), avt_pass(0, 3))
            yield from z_mms(0, 0)
            yield from z_mms(0, 1)
            yield lambda: z_recip(0, 0)
            yield from z_adds(0, 2, nc.vector)
            yield from z_adds(0, 3, nc.vector)
            yield from unT_four(0, 0)
            yield from unT_four(0, 1)
            yield from z_mms(0, 2)
            yield from z_mms(0, 3)
            yield lambda: z_recip(0, 1)
            yield from unT_four(0, 2)
            yield from unT_four(0, 3)
            yield lambda: stage2_pre(0, 0)
            yield lambda: stage2_pre(0, 1)
            yield lambda: stage2_post(0, 0)
            yield lambda: stage2_post(0, 1)

        def out_a_four(b4):
            """4 uc2 transposes -> uf2 with one batched copy."""
            pf = pwk.tile([P, 512], F16, tag="wk", name="pf")
            for t in range(4):
                it = b4 * 4 + t
                nc.tensor.transpose(
                    pf[:, t * P : (t + 1) * P], uc2[:, it], ident
                )
            nc.scalar.copy(
                out=state["uf2"][:, b4 * 512 : (b4 + 1) * 512], in_=pf
            )

        def out_b(it):
            """Out-projection matmul + ysb copy (alternating engines) + DMA."""
            py = pst.tile([P, DIM], F32, tag="st", name="py")
            nc.tensor.matmul(
                py,
                state["uf2"][:, it * P : (it + 1) * P],
                wo_sb,
                start=True,
                stop=True,
            )
            ysb = yp.tile([P, DIM], F16, tag="ysb", name="ysb")
            if it % 2 == 0:
                nc.scalar.copy(out=ysb, in_=py)
                nc.scalar.dma_start(y[it * P : (it + 1) * P, :], ysb)
            else:
                nc.vector.tensor_copy(out=ysb, in_=py)
                nc.gpsimd.dma_start(y[it * P : (it + 1) * P, :], ysb)

        # -------- main per-head jt-loops --------
        trail = None
        for h in range(HPC):
            hs = slice(h * DH, (h + 1) * DH)
            if h == 1:
                trail = trail0()
                state["ET1d"] = xp.tile([P, 4, N], F16, tag="x", name="ET1d")
                state["uf2"] = qp.tile([P, N], F16, tag="q", name="uf2")
            av01 = pav.tile([P, 1024], F32, tag="av", name=f"av01_{h}")
            for jt in range(NT):
                th, lj = et(h, jt)
                for icp in range(2):
                    st = pst.tile([P, 1024], F32, tag="st", name="st")
                    for half in range(2):
                        i0 = icp * 1024 + half * 512
                        nc.tensor.matmul(
                            st[:, half * 512 : (half + 1) * 512],
                            skT[hs, jt * P : (jt + 1) * P],
                            sqT[hs, i0 : i0 + 512],
                            start=True,
                            stop=True,
                            skip_group_check=True,
                        )
                    nc.scalar.activation(
                        th[:, lj, icp * 1024 : (icp + 1) * 1024], st, AF.Exp
                    )
                if jt >= 2:
                    ajt = jt - 2
                    ta, la = et(h, ajt)
                    for c in range(2):
                        nc.tensor.matmul(
                            av01[:, c * 512 : (c + 1) * 512],
                            rv16[:, ajt, :],
                            ta[:, la, c * 512 : (c + 1) * 512],
                            start=(ajt == 0),
                            stop=False,
                            skip_group_check=True,
                        )
                if h == 0:
                    if jt in (1, 3, 5):
                        rv_chunk((jt + 1) // 2)
                    elif jt in (7, 9):
                        rq_chunk((jt - 7) // 2)
                    elif 10 <= jt <= 13:
                        b0 = (jt - 10) * 2
                        rqw_four(b0 // 4, b0 % 4)
                        rqw_four((b0 + 1) // 4, (b0 + 1) % 4)
                        tick(4)
                    else:
                        # sparse jt: keep the PE-activity window busy so the
                        # HAM clock-gate doesn't flap mid-stream
                        tick(8)
                else:
                    ndrain = 12 if jt < 10 else 5
                    for _ in range(ndrain):
                        op = next(trail, None)
                        if op is None:
                            break
                        op()

            def av01_tail(h=h, av01=av01):
                for ajt in (NT - 2, NT - 1):
                    ta, la = et(h, ajt)
                    for c in range(2):
                        nc.tensor.matmul(
                            av01[:, c * 512 : (c + 1) * 512],
                            rv16[:, ajt, :],
                            ta[:, la, c * 512 : (c + 1) * 512],
                            start=False,
                            stop=(ajt == NT - 1),
                            skip_group_check=True,
                        )
                if h == 1:
                    nc.scalar.copy(out=uT16[h][:, 0:1024], in_=av01)
                else:
                    nc.vector.tensor_copy(out=uT16[h][:, 0:1024], in_=av01)

            if h == 0:
                av01_tail()
            else:
                for op in trail:
                    op()
                # avt c2/c3 only read exp cols 1024:2048 and jt<=13 early, so
                # the PE stays busy while the last two exps finish; av01's
                # tail (which reads cols 0:1024 of jt14/15) lands behind it.
                for op in paired(avt_pass(1, 2), avt_pass(1, 3)):
                    op()
                av01_tail()

        # -------- tail: head-1 trailing work pipelined with the
        # out-projection; PE-dense ordering to avoid HAM re-throttle --------
        for op in unT_four(1, 0, on_scalar=True):
            op()
        for op in unT_four(1, 1, on_scalar=True):
            op()
        for op in paired(z_pass16(1, 0), z_pass16(1, 1)):
            op()
        tick(3)
        z_recip(1, 0)
        stage2_pre(1, 0)
        tick(3)
        for op in unT_four(1, 2, on_scalar=True):
            op()
        for op in unT_four(1, 3, on_scalar=True):
            op()
        stage2_post(1, 0)
        tick(3)
        for op in paired(z_pass16(1, 2), z_pass16(1, 3)):
            op()
        z_recip(1, 1)
        tick(4)
        out_a_four(0)
        out_a_four(1)
        stage2_pre(1, 1)
        tick(4)
        for it in range(0, 8):
            out_b(it)
            tick(2)
        stage2_post(1, 1)
        tick(4)
        out_a_four(2)
        out_a_four(3)
        for it in range(8, 16):
            out_b(it)
            tick(2)


def build_program():
    nc = bacc.Bacc(None, target_bir_lowering=False)
    xt = nc.declare_dram_parameter("xt", [DIM, N], F16, isOutput=False)
    wq = nc.declare_dram_parameter("wq", [DIM, P], F16, isOutput=False)
    wk = nc.declare_dram_parameter("wk", [DIM, P], F16, isOutput=False)
    wv = nc.declare_dram_parameter("wv", [DIM, P], F16, isOutput=False)
    wqr = nc.declare_dram_parameter("wqr", [DIM, P], F16, isOutput=False)
    wkt = nc.declare_dram_parameter("wkt", [DH, DH], F16, isOutput=False)
    wo = nc.declare_dram_parameter("wo", [P, DIM], F16, isOutput=False)
    y = nc.declare_dram_parameter("y", [N, DIM], F16, isOutput=True)

    with tile.TileContext(nc) as tc:
        _emit(tc, xt, wq, wk, wv, wqr, wkt, wo, y)
    nc.compile()
    return nc


_NC_CACHE = None


def _get_program():
    global _NC_CACHE
    if _NC_CACHE is None:
        _NC_CACHE = build_program()
    return _NC_CACHE


def make_in_maps(inputs):
    x = np.asarray(inputs["x"], dtype=np.float32)
    wq_s = np.asarray(inputs["wq_s"], dtype=np.float32)
    wk_s = np.asarray(inputs["wk_s"], dtype=np.float32)
    wv_r = np.asarray(inputs["wv_r"], dtype=np.float32)
    wq_r = np.asarray(inputs["wq_r"], dtype=np.float32)
    wk_ret = np.asarray(inputs["wk_ret"], dtype=np.float32)
    w_out = np.asarray(inputs["w_out"], dtype=np.float32)
    scale = np.float32(DH**-0.5)

    f16 = np.float16
    in_maps = []
    for c in range(NCORES):
        b, hp = divmod(c, NCORES // B)
        cols = slice(hp * P, (hp + 1) * P)
        in_maps.append(
            {
                "xt": np.ascontiguousarray(x[b].T).astype(f16),
                "wq": (np.ascontiguousarray(wq_s[:, cols]) * scale).astype(f16),
                "wk": np.ascontiguousarray(wk_s[:, cols]).astype(f16),
                "wv": wv_r.astype(f16),
                "wqr": (np.ascontiguousarray(wq_r[:, cols]) * scale).astype(f16),
                "wkt": np.ascontiguousarray(wk_ret.T).astype(f16),
                "wo": np.ascontiguousarray(w_out[hp * P : (hp + 1) * P, :]).astype(f16),
            }
        )
    return in_maps


def run(inputs, trace=False, **kw):
    res = run_bass_kernel_spmd(
        _get_program(), make_in_maps(inputs), list(range(NCORES)), trace=trace, **kw
    )
    out = np.zeros((B, N, DIM), np.float32)
    for c in range(NCORES):
        out[c // (NCORES // B)] += np.asarray(res.results[c]["y"], np.float32)
    return out, res


def kernel(**inputs):
    out, _ = run(inputs)
    return out
